# revision 81
# baseline (speedup 1.0000x reference)
"""Chunked-causal GQA attention (B=2, S=2048, Hq=16, Hkv=8, D=128, chunk=512)
on 8 TRN2 NeuronCores.

Sharding: the 16 (batch, kv_head) pairs are split 2-per-core. Each pair is a
fully independent attention problem (its 2 query heads attend to its K/V),
so there are no cross-core collectives and every HBM byte is read once.

Per-core dataflow (per chunk of 512 tokens, per kv pair):
  - Plain f32 DMA loads Q/K/V with the sequence dim folded onto partitions.
  - TensorE transpose (f32, via identity matmul) produces Q^T/K^T with the
    head dim on partitions; the PSUM->SBUF evacuation casts to bf16.
  - QK^T is computed *transposed*: logitsT[k, q] = K^T_j.T @ Q^T, so the
    softmax numerator P^T comes out of exp() already laid out as the
    stationary operand of the PV matmul (no P transpose needed).
    Block-causal structure skips fully-masked blocks (37.5% of the work).
  - The causal mask inside each diagonal 128x128 block is folded into the
    QK PSUM accumulation: a chained matmul adds a constant strictly-lower
    -1e38 triangle, so exp() emits exact zeros there with no extra pass.
  - ScalarE computes exp(logits * 1/sqrt(D)), PSUM f32 -> SBUF bf16.
  - PV: out[q, :] = sum_j P^T_j.T @ [V_j | 1]; the appended ones column
    makes the softmax row-sum fall out of the same matmul. DVE multiplies
    by the reciprocal row-sum to normalize.
  - An explicit software pipeline emits the next chunk's transposes between
    this chunk's QK groups so the PE stays busy during exp() latency. Two
    1280-float logit slots pack into one 5-PSUM-bank tile (only the middle
    bank is shared, so its j2 block issues last), double-buffering the
    QK -> exp chain within the 8-bank budget; a PE warm-up burst during the
    first loads ramps the HAM clock gate before the first real matmuls.
"""

import math
import numpy as np
from contextlib import ExitStack

_BUILT = None

B, S, HQ, HKV, D = 2, 2048, 16, 8, 128
C = 512          # chunk size
NCH = S // C     # chunks per sequence
NB = C // 128    # 128-blocks per chunk


def _build():
    from concourse import bacc, tile, mybir

    f32 = mybir.dt.float32
    f32r = mybir.dt.float32r
    bf16 = mybir.dt.bfloat16
    Exp = mybir.ActivationFunctionType.Exp
    mult = mybir.AluOpType.mult

    nc = bacc.Bacc("TRN2", target_bir_lowering=False, debug=False, num_devices=8)

    q_d = nc.dram_tensor("query", [S, 4, D], f32, kind="ExternalInput").ap()
    k_d = nc.dram_tensor("key", [S, 2, D], f32, kind="ExternalInput").ap()
    v_d = nc.dram_tensor("value", [S, 2, D], f32, kind="ExternalInput").ap()
    tri_d = nc.dram_tensor("tri", [128, 128], f32, kind="ExternalInput").ap()
    idn_d = nc.dram_tensor("ident", [128, 128], f32, kind="ExternalInput").ap()
    o_d = nc.dram_tensor("out", [S, 4, D], f32, kind="ExternalOutput").ap()

    SCALE = 1.0 / math.sqrt(D)
    # logitsT block j (k-block j vs q in [128j, 512)) start offsets. Two
    # 1280-f32 logit slots (A/B) pack into one 5-bank PSUM tile so that
    # consecutive QK groups double-buffer; only bank 2 (the two j2 blocks)
    # is shared, so j2 is issued last in each group.
    OFFS = [
        {0: 0, 1: 512, 3: 896, 2: 1024},
        {2: 1280, 0: 1536, 1: 2048, 3: 2432},
    ]
    W = {j: C - 128 * j for j in range(4)}
    # PV output tile i (per head-group) lives in bank i//3 at slot i%3
    # (stride 130 keeps slots 8-byte aligned); col 128 is the row-sum.
    PO = {i: (i // 3) * 512 + (i % 3) * 130 for i in range(4)}

    with tile.TileContext(nc) as tc, ExitStack() as ctx:
        cstp = ctx.enter_context(tc.tile_pool(name="consts", bufs=1))
        qp = ctx.enter_context(tc.tile_pool(name="qnat", bufs=4))
        kp = ctx.enter_context(tc.tile_pool(name="knat", bufs=4))
        vrp = ctx.enter_context(tc.tile_pool(name="vraw", bufs=4))
        vp = ctx.enter_context(tc.tile_pool(name="vt", bufs=4))
        qtp = ctx.enter_context(tc.tile_pool(name="qt", bufs=3))
        ktp = ctx.enter_context(tc.tile_pool(name="kt", bufs=3))
        ptp = ctx.enter_context(tc.tile_pool(name="pt", bufs=3))
        rp = ctx.enter_context(tc.tile_pool(name="recip", bufs=4))
        op = ctx.enter_context(tc.tile_pool(name="osb", bufs=4))
        tps = ctx.enter_context(tc.tile_pool(name="tps", bufs=1, space="PSUM"))
        lgp = ctx.enter_context(tc.tile_pool(name="lg", bufs=1, space="PSUM"))
        pop = ctx.enter_context(tc.tile_pool(name="po", bufs=1, space="PSUM"))

        tri_f = cstp.tile([128, 128], f32)
        idn_sb = cstp.tile([128, 128], f32)
        trineg_sb = cstp.tile([128, 128], bf16)
        idn_bf = cstp.tile([128, 128], bf16)
        nc.gpsimd.dma_start(out=tri_f, in_=tri_d)
        nc.vector.tensor_copy(trineg_sb, tri_f)

        chunks = [(p, n) for p in range(2) for n in range(NCH)]
        lg2 = lgp.tile([128, 2560], f32, name="lg2")  # 5 banks, slots A/B

        def transpose_q(st, g):
            # one 1-bank round: 4 Q blocks of group g -> qt[:, g*C:(g+1)*C]
            q_c, _ = st["ld"]
            if g == 0:
                st["qt"] = qtp.tile([128, 2 * C], bf16, name="qt")  # [d, (g, q)]
            qt = st["qt"]
            t = tps.tile([128, 4, 128], f32r, tag="tps")  # 1 bank
            for qb in range(NB):
                nc.tensor.transpose(
                    t[:, qb, :], q_c[:, qb, g, :].bitcast(f32r), idn_sb.bitcast(f32r)
                )
            nc.vector.tensor_copy(qt[:, g * C : (g + 1) * C], t[:, 0:4, :])

        def transpose_k(st):
            k_c = st["k_c"]
            st["kt"] = ktp.tile([128, C], bf16, name="kt")      # [d, k]
            t = tps.tile([128, 4, 128], f32r, tag="tps")  # 1 bank
            for kb in range(NB):
                nc.tensor.transpose(
                    t[:, kb, :], k_c[:, kb, :].bitcast(f32r), idn_sb.bitcast(f32r)
                )
            st["kt_ps"] = t

        def transpose_k_copy(st):
            nc.scalar.copy(st["kt"], st["kt_ps"][:, 0:4, :])

        def qk_exp(st, g, slot):
            qt, kt, pt = st["qt"], st["kt"], st["pt"]
            OFF = OFFS[slot]
            if True:
                # Pairs (0,1) then (3,2): QK for both blocks, then both causal
                # mask-adds back to back, so the trineg stationary loads once
                # per pair instead of alternating with kt every matmul. Bank
                # legality: j0/j1 open groups in different banks; j3 reuses
                # j1's bank only after its mask-add closed the group; j2 last
                # (its bank is shared across the A/B slots).
                for ja, jb in ((0, 1), (3, 2)):
                    for j in (ja, jb):
                        nc.tensor.matmul(
                            lg2[:, OFF[j] : OFF[j] + W[j]],
                            lhsT=kt[:, j * 128 : (j + 1) * 128],
                            rhs=qt[:, g * C + 128 * j : (g + 1) * C],
                            start=True,
                            stop=False,
                        )
                    for j in (ja, jb):
                        # add -1e38 to the below-diagonal half of the diagonal
                        # 128x128 region (trineg.T is strictly-lower in [k, q])
                        nc.tensor.matmul(
                            lg2[:, OFF[j] : OFF[j] + 128],
                            lhsT=trineg_sb,
                            rhs=idn_bf,
                            start=False,
                            stop=True,
                        )
                base = g * 1280
                nc.scalar.activation(
                    pt[:, base : base + 1280],
                    lg2[:, slot * 1280 : (slot + 1) * 1280],
                    Exp,
                    scale=SCALE,
                )

        # pt column layout per group mirrors its lg slot's block order
        PTOFF = [
            {0: 0, 1: 512, 3: 896, 2: 1024},
            {2: 0, 0: 256, 1: 768, 3: 1152},
        ]

        def pv(st, g):
            pt = st["pt"]
            _, v_t = st["ld"]
            po = pop.tile([128, 1024], f32, name="po")  # 2 banks (4 slots)
            st["po" + str(g)] = po
            base = g * 1280
            for i in range(NB):
                oap = po[:, PO[i] :][:, 0 : D + 1]
                for j in range(i + 1):
                    nc.tensor.matmul(
                        oap,
                        lhsT=pt[:, base + PTOFF[g][j] + (i - j) * 128 :][:, 0:128],
                        rhs=v_t[:, j, 0 : D + 1],
                        start=(j == 0),
                        stop=(j == i),
                    )

        def finish(st, g, split_store=False):
            p, n = st["pn"]
            po = st["po" + str(g)]
            osb = op.tile([128, NB, D], f32)
            recip = rp.tile([128, 4], f32)
            dst = o_d[n * C : (n + 1) * C, 2 * p + g, :].rearrange(
                "(i qp) d -> qp i d", qp=128
            )
            for b in range(2):
                cnt = 3 if b == 0 else 1
                blk = po[:, b * 512 : b * 512 + cnt * 130].rearrange(
                    "p (c w) -> p c w", w=130
                )
                nc.vector.reciprocal(recip[:, 3 * b : 3 * b + cnt], blk[:, :, 128:129])
                rb = (
                    recip[:, 3 * b : 3 * b + cnt]
                    .unsqueeze(2)
                    .broadcast_to((128, cnt, 128))
                )
                nc.vector.tensor_tensor(
                    osb[:, 3 * b : 3 * b + cnt, :], blk[:, :, 0:128], rb, op=mult
                )
                if split_store:
                    # tail only: ship each bank's rows as soon as normalized
                    nc.sync.dma_start(
                        out=dst[:, 3 * b : 3 * b + cnt, :],
                        in_=osb[:, 3 * b : 3 * b + cnt, :],
                    )
            if not split_store:
                nc.sync.dma_start(out=dst, in_=osb)

        def stage_in(i):
            """Allocate tiles + issue loads for chunk i."""
            p, n = chunks[i]
            st = {"pn": (p, n)}
            s0 = n * C
            q_c = qp.tile([128, NB, 2, D], f32)
            k_c = kp.tile([128, NB, D], f32)
            v_raw = vrp.tile([128, NB, D], f32)
            v_t = vp.tile([128, NB, 132], bf16)
            # load order matches transpose-round consumption: q-g0, k, q-g1, v
            nc.sync.dma_start(
                out=q_c[:, :, 0, :],
                in_=q_d[s0 : s0 + C, 2 * p, :].rearrange(
                    "(b qp) d -> qp b d", qp=128
                ),
            )
            if i == 0:
                # identity const rides second on the lane: its transfer is
                # small, so the first transposes get both inputs earliest
                nc.sync.dma_start(out=idn_sb, in_=idn_d)
            nc.sync.dma_start(
                out=k_c,
                in_=k_d[s0 : s0 + C, p, :].rearrange("(b kp) d -> kp b d", kp=128),
            )
            nc.sync.dma_start(
                out=q_c[:, :, 1, :],
                in_=q_d[s0 : s0 + C, 2 * p + 1, :].rearrange(
                    "(b qp) d -> qp b d", qp=128
                ),
            )
            nc.sync.dma_start(
                out=v_raw,
                in_=v_d[s0 : s0 + C, p, :].rearrange("(b kp) d -> kp b d", kp=128),
            )
            st["v_raw"] = v_raw
            st["ld"] = (q_c, v_t)
            st["k_c"] = k_c
            return st

        def vconv(st):
            _, v_t = st["ld"]
            nc.vector.tensor_copy(v_t[:, :, 0:D], st["v_raw"])
            nc.gpsimd.memset(v_t[:, :, D : D + 1], 1.0)

        # Software pipeline: next chunk's transposes fill the PE while this
        # chunk's exp() runs on ScalarE.
        # PE warm-up: tiny matmuls during the first loads ramp the PE
        # p-state (HAM) so the first chunk's transposes/QK run at full clock.
        # Fed from a memset tile so it starts before any DMA lands.
        wsrc = cstp.tile([128, 128], f32)
        nc.vector.memset(wsrc, 1.0)
        tw = tps.tile([128, 4, 128], f32, tag="tps")
        for r in range(56):
            nc.tensor.matmul(
                tw[:, 0, r : r + 1], lhsT=wsrc, rhs=wsrc[:, r : r + 1],
                start=True, stop=True,
            )

        sts = {}
        sts[0] = stage_in(0)
        nc.vector.tensor_copy(idn_bf, idn_sb)  # after the hoisted idn DMA
        vconv(sts[0])
        transpose_q(sts[0], 0)
        transpose_k(sts[0])
        transpose_k_copy(sts[0])
        transpose_q(sts[0], 1)
        prev = None
        for i in range(len(chunks)):
            st = sts[i]
            st["pt"] = ptp.tile([128, 2560], bf16, name="pt")
            nxt = None
            if i + 1 < len(chunks):
                nxt = sts[i + 1] = stage_in(i + 1)
            qk_exp(st, 0, 0)
            if prev is not None:
                pv(prev, 1)       # deferred from the previous iteration so
                finish(prev, 1)   # QK g0 leads the PE stream
            if nxt is not None:
                transpose_q(nxt, 0)
                transpose_k(nxt)
            qk_exp(st, 1, 1)
            if nxt is not None:
                transpose_k_copy(nxt)
            pv(st, 0)
            finish(st, 0)
            if nxt is not None:
                transpose_q(nxt, 1)
            if nxt is not None:
                vconv(nxt)
            if prev is not None:
                del sts[i - 1]
            prev = st
        pv(prev, 1)
        finish(prev, 1, split_store=True)

    nc.compile()
    return nc


def _get_nc():
    global _BUILT
    if _BUILT is None:
        _BUILT = _build()
    return _BUILT


def _consts():
    # "tri" is the causal mask-bias: out[k,q] += tri.T[k,q]; strictly-upper
    # -1e38 in [q,k] orientation masks q < k after the transpose-by-matmul.
    tri = np.triu(np.full((128, 128), -1.0e38, np.float32), k=1)
    idn = np.eye(128, dtype=np.float32)
    return tri, idn


def kernel(query, key, value, chunk_size):
    from concourse.bass_utils import run_bass_kernel_spmd

    assert int(chunk_size) == C
    q = np.ascontiguousarray(np.asarray(query, dtype=np.float32))
    k = np.ascontiguousarray(np.asarray(key, dtype=np.float32))
    v = np.ascontiguousarray(np.asarray(value, dtype=np.float32))
    tri, idn = _consts()

    nc = _get_nc()
    in_maps = []
    for c in range(8):
        b, h0 = divmod(2 * c, HKV)
        in_maps.append(
            {
                "query": np.ascontiguousarray(q[b, :, 2 * h0 : 2 * h0 + 4, :]),
                "key": np.ascontiguousarray(k[b, :, h0 : h0 + 2, :]),
                "value": np.ascontiguousarray(v[b, :, h0 : h0 + 2, :]),
                "tri": tri,
                "ident": idn,
            }
        )
    res = run_bass_kernel_spmd(nc, in_maps, core_ids=list(range(8)))
    out = np.empty((B, S, HQ, D), np.float32)
    for c in range(8):
        b, h0 = divmod(2 * c, HKV)
        out[b, :, 2 * h0 : 2 * h0 + 4, :] = res.results[c]["out"]
    return out
